# revision 1
# baseline (speedup 1.0000x reference)
"""Trainium2 Bass kernel for nn_AFW_63118839382657 (low-rank cross-modal bilinear net).

Key algebraic identity: G = (q1 outer q2) * (k1 outer k2) = (q1*k1) outer (q2*k2),
i.e. G is rank-1 per (b, t) with factors u = q1*k1, v = q2*k2 in R^32.
Everything then reduces to:
  Mqk[b]   = u[b].T @ v[b] / T                      (tiny matmuls)
  P_j[b]   = Mqk[m1] @ Mqk[m2]                      (32x32 matmuls)
  w[b]     = v[b] @ P_j[b]                          (w.T = P.T @ v.T)
  proj     = Z @ Watt,  Z[bt, k*32+o] = u[bt,k]*w[bt,o]
  out      = (proj + batt + beta) * X
Z.T lives on the contraction partitions for the big matmul; it is built as
urep (u rows broadcast via a DRAM-roundtrip replicated read) * wrep (w tiled 4x
along partitions via SBUF partition-shift copies), multiplied on VectorE in bf16.

Data-parallel over batch: B=16 -> 2 per core across 8 NeuronCores, weights
replicated, no collectives. All host-side layout prep (transposes, weight
stacking, bf16 conversion of Watt) happens in kernel() below.

Precision: projections in fp32r (FP22, full PE rate), u/v/w/Z in bf16, Watt in
bf16, X and outputs in fp32. Measured absmax relative error vs the fp32 jax
reference: 4.1e-4. CoreSim cost-model kernel time: ~120.9 us/core (PE ~83% busy;
pure-matmul floor ~92 us). Emission is software-pipelined: layer-2 stage-1 is
interleaved into layer-1's big-matmul blocks, DMA transfers are spread across
the SP/ACT HWDGE rings and gpsimd SWDGE, a PE warm-up burst covers the HAM
cold-start window, and the q/k partition alignment uses PE identity-matmul
shifts instead of DMA hops on latency-critical chains.
"""
import sys

for _p in ("/opt/trn_rl_repo", "/opt/pypackages"):
    if _p not in sys.path:
        sys.path.insert(0, _p)

import numpy as np
import ml_dtypes
from contextlib import ExitStack

import concourse.bass as bass
import concourse.mybir as mybir
import concourse.tile as tile
from concourse import bacc
from concourse.masks import make_identity
from concourse.bass_utils import run_bass_kernel_spmd

F32 = mybir.dt.float32
F32R = mybir.dt.float32r
BF16 = mybir.dt.bfloat16
Copy = mybir.ActivationFunctionType.Copy
Ident = mybir.ActivationFunctionType.Identity

L, MODS, D, R, B, T = 2, 3, 512, 32, 16, 512
BETA = 0.1
NCORES = 8
BL = B // NCORES          # local batch = 2
BT = BL * T               # 1024
P = 128
KC = D // P               # 4 d-chunks (projection contraction)
KO = (R * R) // P         # 8 ko-chunks (big matmul contraction)
DT = D // P               # 4 d-tiles (big matmul output)
NT = BT // 512            # 2 bt-halves

# layout/scheduling knobs (tuned via CoreSim cost-model sweeps)
CFG = dict(
    wt_evac="act",      # engine for w.T psum->sbuf evac: "dve" | "act"
    psq_split=True,     # per-nt [128,512] projection psums
    ps_big_bufs=4,
    big_n1024=False,
    ps_sm_bufs=2,
)


def build_graph(nc):
    xt = nc.dram_tensor("xt", [MODS, D, BT], F32R, kind="ExternalInput").ap()
    wqk = nc.dram_tensor("wqk", [L * MODS, D, 128], F32R, kind="ExternalInput").ap()
    bqk = nc.dram_tensor("bqk", [L * MODS, 128], F32, kind="ExternalInput").ap()
    watt = nc.dram_tensor("watt", [L * MODS, R * R, D], BF16, kind="ExternalInput").ap()
    bout = nc.dram_tensor("bout", [L * MODS, D], F32, kind="ExternalInput").ap()
    out = nc.dram_tensor("out", [MODS, D, BT], F32, kind="ExternalOutput").ap()

    with tile.TileContext(nc) as tc, ExitStack() as ctx:
        const = ctx.enter_context(tc.tile_pool(name="const", bufs=1))
        xpool = ctx.enter_context(tc.tile_pool(name="xpool", bufs=16))
        wattp = ctx.enter_context(tc.tile_pool(name="wattp", bufs=2))
        qkp = ctx.enter_context(tc.tile_pool(name="qkp", bufs=1))
        kp = ctx.enter_context(tc.tile_pool(name="kp", bufs=2))
        uvp = ctx.enter_context(tc.tile_pool(name="uvp", bufs=4))
        natp = ctx.enter_context(tc.tile_pool(name="natp", bufs=2))
        mp = ctx.enter_context(tc.tile_pool(name="mp", bufs=8))
        pp_ = ctx.enter_context(tc.tile_pool(name="pp", bufs=6))
        wrp = ctx.enter_context(tc.tile_pool(name="wrp", bufs=2))
        urp = ctx.enter_context(tc.tile_pool(name="urp", bufs=14))
        zp = ctx.enter_context(tc.tile_pool(name="zp", bufs=16))
        rp = ctx.enter_context(tc.tile_pool(name="rp", bufs=3))
        op_ = ctx.enter_context(tc.tile_pool(name="op", bufs=3))
        dramp = ctx.enter_context(tc.tile_pool(name="dramp", bufs=4, space="DRAM"))

        ps_qk = ctx.enter_context(tc.tile_pool(name="ps_qk", bufs=2 if CFG["psq_split"] else 1, space="PSUM"))
        ps_sm = ctx.enter_context(tc.tile_pool(name="ps_sm", bufs=CFG["ps_sm_bufs"], space="PSUM"))
        ps_big = ctx.enter_context(tc.tile_pool(name="ps_big", bufs=CFG["ps_big_bufs"], space="PSUM"))

        # constants: first modality's weights first, X.T chunked across both
        # HWDGE rings so the first projection matmul can start early.
        wqk_sb = const.tile([P, L * MODS, KC, P], F32R)
        bqk_sb = const.tile([P, L * MODS], F32)
        bout_sb = const.tile([P, L * MODS, DT], F32)
        nc.sync.dma_start(
            out=wqk_sb[:, 0, :, :], in_=wqk[0].rearrange("(c p) w -> p c w", p=P)
        )
        xt_cur = [[None] * KC for _ in range(MODS)]
        rings = {
            0: [nc.scalar, nc.sync, nc.scalar, nc.sync],
            1: [nc.gpsimd, nc.sync, nc.gpsimd, nc.sync],
            2: [nc.gpsimd, nc.sync, nc.gpsimd, nc.sync],
        }
        for m in range(MODS):
            xv = xt[m].rearrange("(c p) bt -> p c bt", p=P)
            for c in range(KC):
                xmc = xpool.tile([P, BT], F32R, tag="x", name=f"xm{m}c{c}")
                rings[m][c].dma_start(out=xmc, in_=xv[:, c, :])
                xt_cur[m][c] = xmc
            if m + 1 < MODS:
                nc.sync.dma_start(
                    out=wqk_sb[:, m + 1, :, :],
                    in_=wqk[m + 1].rearrange("(c p) w -> p c w", p=P),
                )
        for lm in range(MODS, L * MODS):
            nc.sync.dma_start(
                out=wqk_sb[:, lm, :, :],
                in_=wqk[lm].rearrange("(c p) w -> p c w", p=P),
            )
        for lm in range(L * MODS):
            nc.scalar.dma_start(
                out=bqk_sb[:, lm : lm + 1],
                in_=bqk[lm].rearrange("(p o) -> p o", o=1),
            )
        ident = const.tile([64, 64], BF16)
        make_identity(nc, ident)
        identh = const.tile([128, 64], BF16)
        make_identity(nc, identh[64:128, :])
        wup = ps_big.tile([64, 64], BF16, tag="big", name="wup")
        for _ in range(56):
            nc.tensor.transpose(wup, ident, ident)


        # Per-layer state, keyed by layer index.
        S = {
            li: dict(Ms={}, Ps={}, uvTs=[None] * MODS, ut_dr=[None] * MODS,
                     watt_sb=[None] * MODS, wreps=[None] * MODS,
                     ureps=[None] * MODS, zTs=[None] * MODS)
            for li in range(L)
        }

        def s1(li, m, kt_pe=False):
            """Projections, u/v factors, Mqk forms for (layer, modality)."""
            st = S[li]
            lm = li * MODS + m
            qkT = qkp.tile([P, BT], BF16, tag="qkT", name=f"qkT{lm}")
            uvT = uvp.tile([64, BT], BF16, tag="uvT", name=f"uvT{lm}")
            kparts = []
            for nt in range(NT):
                sl = slice(nt * 512, (nt + 1) * 512)
                psq = ps_qk.tile([P, 512], F32, tag="qk", name=f"psq{nt}")
                for c in range(KC):
                    nc.tensor.matmul(
                        psq,
                        lhsT=wqk_sb[:, lm, c, :],
                        rhs=xt_cur[m][c][:, sl],
                        start=(c == 0),
                        stop=(c == KC - 1),
                    )
                nc.scalar.activation(
                    out=qkT[:, sl], in_=psq, func=Ident,
                    bias=bqk_sb[:, lm : lm + 1],
                )
                if kt_pe:
                    # shift k-rows to partitions 0:64 via identity matmul
                    # (cuts the SBUF-shift DMA latency on the layer boundary)
                    psk = ps_sm.tile([64, 512], F32, tag="small", name=f"psk{nt}")
                    nc.tensor.matmul(
                        psk,
                        lhsT=identh[64:128, :],
                        rhs=qkT[64:128, sl],
                        tile_position=(64, 0),
                    )
                    nc.vector.tensor_mul(
                        out=uvT[:, sl], in0=qkT[0:64, sl], in1=psk
                    )
            if not kt_pe:
                ktile = kp.tile([64, BT], BF16, tag="kT", name=f"kt{lm}")
                nc.scalar.dma_start(out=ktile, in_=qkT[64:128, :])
                nc.vector.tensor_mul(out=uvT, in0=qkT[0:64, :], in1=ktile)
            st["uvTs"][m] = uvT

            ud = dramp.tile([32, BT], BF16, tag="ut", name=f"ud{lm}")
            ud_inst = nc.gpsimd.dma_start(out=ud, in_=uvT[0:32, :])
            st["ut_dr"][m] = ud

            uv_nat = natp.tile([P, KO, 64], BF16, tag="nat", name=f"nat{lm}")
            for c8 in range(KO):
                pst = ps_sm.tile([P, 64], BF16, tag="small", name=f"pst{c8}")
                nc.tensor.transpose(pst, uvT[:, c8 * P : (c8 + 1) * P], ident)
                nc.vector.tensor_copy(out=uv_nat[:, c8, :], in_=pst)

            forms = []
            if m in (0, 1):
                forms.append("L")
            if m in (1, 2):
                forms.append("R")
            slots = [(b, f) for b in range(BL) for f in forms]
            pm = ps_sm.tile([32, len(slots), 32], F32, tag="small", name="pm")
            for si, (b, f) in enumerate(slots):
                for cc in range(4):
                    ch = b * 4 + cc
                    if f == "L":
                        lhs = uv_nat[:, ch, 32:64]
                        rhs = uv_nat[:, ch, 0:32]
                    else:
                        lhs = uv_nat[:, ch, 0:32]
                        rhs = uv_nat[:, ch, 32:64]
                    nc.tensor.matmul(
                        pm[:, si, :], lhsT=lhs, rhs=rhs,
                        start=(cc == 0), stop=(cc == 3),
                    )
            msb = mp.tile([32, len(slots), 32], BF16, tag="m", name=f"M{m}")
            nc.vector.tensor_copy(out=msb, in_=pm)
            for si, (b, f) in enumerate(slots):
                st["Ms"][(f, m, b)] = msb[:, si, :]
            wsb = wattp.tile([P, KO, D], BF16, tag="watt", name=f"wsb{lm}")
            wsb_inst = nc.gpsimd.dma_start(
                out=wsb, in_=watt[lm].rearrange("(c p) d -> p c d", p=P)
            )
            # ordering-only hint: keep the bulky watt load behind the
            # latency-critical x/u traffic on the POOL ring
            tile.add_dep_helper(
                wsb_inst.ins,
                ud_inst.ins,
                sync=False,
                reason="watt load after ud on POOL",
            )
            st["watt_sb"][m] = wsb

        def pblock(li, js=range(MODS)):
            """Cross-modal P products; emits P tiled 4x along free dim so the
            w-matmul can write the partition-replicated wrep directly."""
            st = S[li]
            for j in js:
                for b in range(BL):
                    m1, m2 = [x for x in range(MODS) if x != j]
                    rhs4 = st["Ms"][("R", m2, b)][:, None, :].to_broadcast((32, 4, 32))
                    pps = ps_sm.tile([64, 4, 32], F32, tag="small", name=f"pps{j}{b}")
                    nc.tensor.matmul(
                        pps[32:64],
                        lhsT=st["Ms"][("L", m1, b)],
                        rhs=rhs4,
                        tile_position=(0, 32),
                    )
                    ph = pp_.tile([64, 4, 32], BF16, tag="p", name=f"ph{j}{b}")
                    nc.vector.tensor_scalar_mul(
                        ph[32:64], pps[32:64], 1.0 / (T * T)
                    )
                    st["Ps"][(j, b)] = ph

        def prep(li, m):
            """urep broadcast loads + direct partition-replicated w (wrep)."""
            st = S[li]
            urep = []
            for c in range(KO):
                uc = urp.tile([P, BT], BF16, tag="urep", name=f"ur{li}{m}c{c}")
                src_ap = st["ut_dr"][m][4 * c : 4 * c + 4][:, None, :].to_broadcast(
                    (4, 32, BT)
                )
                nc.sync.dma_start(out=uc, in_=src_ap)
                urep.append(uc)
            st["ureps"][m] = urep
            wrep = wrp.tile([P, BT], BF16, tag="wrep", name=f"wrep{li}{m}")
            for b in range(BL):
                pw = ps_sm.tile([P, 512], F32, tag="small", name=f"pw{m}{b}")
                nc.tensor.matmul(
                    pw,
                    lhsT=st["Ps"][(m, b)][32:64].rearrange("p a b -> p (a b)"),
                    rhs=st["uvTs"][m][32:64, b * 512 : (b + 1) * 512],
                    tile_position=(32, 0),
                )
                if CFG["wt_evac"] == "dve":
                    nc.vector.tensor_copy(
                        out=wrep[:, b * 512 : (b + 1) * 512], in_=pw
                    )
                else:
                    nc.scalar.activation(
                        out=wrep[:, b * 512 : (b + 1) * 512], in_=pw, func=Copy
                    )
            st["wreps"][m] = wrep

        def zmuls(li, m):
            st = S[li]
            zT = []
            for c in range(KO):
                zc = zp.tile([P, BT], BF16, tag="zT", name=f"z{li}{m}c{c}")
                for b in range(BL):
                    hs = slice(b * 512, (b + 1) * 512)
                    nc.vector.tensor_mul(
                        out=zc[:, hs],
                        in0=st["ureps"][m][c][:, hs],
                        in1=st["wreps"][m][:, hs],
                    )
                zT.append(zc)
            st["zTs"][m] = zT

        def big(li, m):
            """Big matmul proj.T = Watt.T @ Z.T + residual combine for (layer, mod)."""
            st = S[li]
            lm = li * MODS + m
            zT = st["zTs"][m]
            if li == 0:
                xnew = [
                    xpool.tile([P, BT], F32R, tag="x", name=f"xn{m}c{c}")
                    for c in range(KC)
                ]
            else:
                outm = out[m].rearrange("(t p) bt -> p t bt", p=P)
            if CFG["big_n1024"]:
                for dt_i in range(DT):
                    pb = ps_big.tile([P, BT], F32, tag="big", name="pb")
                    for c in range(KO):
                        nc.tensor.matmul(
                            pb,
                            lhsT=st["watt_sb"][m][:, c, dt_i * P : (dt_i + 1) * P],
                            rhs=zT[c],
                            start=(c == 0),
                            stop=(c == KO - 1),
                        )
                    res = rp.tile([P, BT], F32, tag="res")
                    nc.scalar.activation(
                        out=res,
                        in_=pb,
                        func=Ident,
                        bias=bout_sb[:, lm, dt_i : dt_i + 1],
                    )
                    if li == 0:
                        nc.vector.tensor_mul(
                            out=xnew[dt_i],
                            in0=res,
                            in1=xt_cur[m][dt_i].bitcast(F32),
                        )
                    else:
                        ost = op_.tile([P, BT], F32, tag="ost")
                        nc.vector.tensor_mul(
                            out=ost,
                            in0=res,
                            in1=xt_cur[m][dt_i].bitcast(F32),
                        )
                        (nc.sync if dt_i % 2 == 0 else nc.scalar).dma_start(
                            out=outm[:, dt_i, :], in_=ost
                        )
                continue_marker = True
            else:
                for dt_i in range(DT):
                    pbig = [
                        ps_big.tile([P, 512], F32, tag="big", name=f"pbig{nt}")
                        for nt in range(NT)
                    ]
                    for c in range(KO):
                        for nt in range(NT):
                            nc.tensor.matmul(
                                pbig[nt],
                                lhsT=st["watt_sb"][m][:, c, dt_i * P : (dt_i + 1) * P],
                                rhs=zT[c][:, nt * 512 : (nt + 1) * 512],
                                start=(c == 0),
                                stop=(c == KO - 1),
                            )
                    for nt in range(NT):
                        sl = slice(nt * 512, (nt + 1) * 512)
                        res = rp.tile([P, 512], F32, tag="res")
                        nc.scalar.activation(
                            out=res,
                            in_=pbig[nt],
                            func=Ident,
                            bias=bout_sb[:, lm, dt_i : dt_i + 1],
                        )
                        if li == 0:
                            nc.vector.tensor_mul(
                                out=xnew[dt_i][:, sl],
                                in0=res,
                                in1=xt_cur[m][dt_i][:, sl].bitcast(F32),
                            )
                        else:
                            ost = op_.tile([P, 512], F32, tag="ost")
                            for hh in range(2):
                                hs = slice(hh * 256, (hh + 1) * 256)
                                gs = slice(nt * 512 + hh * 256, nt * 512 + (hh + 1) * 256)
                                nc.vector.tensor_mul(
                                    out=ost[:, hs],
                                    in0=res[:, hs],
                                    in1=xt_cur[m][dt_i][:, gs].bitcast(F32),
                                )
                                (nc.sync if (nt + hh) % 2 == 0 else nc.scalar).dma_start(
                                    out=outm[:, dt_i, gs], in_=ost[:, hs]
                                )
            if li == 0:
                xt_cur[m] = xnew

        # ---- software-pipelined emission: layer-2 stage-1 hides under
        # ---- layer-1 big matmuls.
        for m in range(MODS):
            s1(0, m, kt_pe=True)
        pblock(0, js=(0,))
        prep(0, 0)
        zmuls(0, 0)
        pblock(0, js=(1,))
        prep(0, 1)
        zmuls(0, 1)
        pblock(0, js=(2,))
        prep(0, 2)
        for lm in range(L * MODS):
            nc.gpsimd.dma_start(
                out=bout_sb[:, lm, :],
                in_=bout[lm].rearrange("(t p) -> p t", p=P),
            )
        big(0, 0)
        s1(1, 0)
        zmuls(0, 2)
        big(0, 1)
        s1(1, 1)
        big(0, 2)
        s1(1, 2, kt_pe=True)
        pblock(1, js=(0,))
        prep(1, 0)
        zmuls(1, 0)
        pblock(1, js=(1,))
        prep(1, 1)
        zmuls(1, 1)
        pblock(1, js=(2,))
        prep(1, 2)
        big(1, 0)
        zmuls(1, 2)
        big(1, 1)
        big(1, 2)

    nc.finalize()
    return nc


_NC_CACHE = None


def _get_nc():
    global _NC_CACHE
    if _NC_CACHE is None:
        nc = bacc.Bacc("TRN2", target_bir_lowering=False, debug=False)
        _NC_CACHE = build_graph(nc)
    return _NC_CACHE


def make_in_maps(inputs):
    wqk = np.concatenate(
        [inputs["Wq1"], inputs["Wq2"], inputs["Wk1"], inputs["Wk2"]], axis=-1
    ).reshape(L * MODS, D, 128)
    bqk = np.concatenate(
        [inputs["bq1"], inputs["bq2"], inputs["bk1"], inputs["bk2"]], axis=-1
    ).reshape(L * MODS, 128).astype(np.float32)
    watt = np.asarray(inputs["Watt"], np.float32).reshape(L * MODS, R * R, D)
    watt_bf = watt.astype(ml_dtypes.bfloat16)
    bout = (np.asarray(inputs["batt"], np.float32) + np.float32(BETA)).reshape(
        L * MODS, D
    )
    xs = [np.asarray(inputs[k], np.float32) for k in ("x_a", "x_t", "x_v")]
    in_maps = []
    for core in range(NCORES):
        sh = slice(core * BL, (core + 1) * BL)
        xts = np.stack(
            [np.ascontiguousarray(x[sh].reshape(BT, D).T) for x in xs]
        ).astype(np.float32)
        in_maps.append(
            {
                "xt": xts,
                "wqk": np.ascontiguousarray(wqk, dtype=np.float32),
                "bqk": bqk,
                "watt": watt_bf,
                "bout": bout,
            }
        )
    return in_maps


def assemble(results):
    full = [np.empty((B, T, D), np.float32) for _ in range(MODS)]
    for core in range(NCORES):
        o = results[core]["out"]  # [MODS, D, BT]
        for m in range(MODS):
            full[m][core * BL : (core + 1) * BL] = (
                o[m].T.reshape(BL, T, D).astype(np.float32)
            )
    return tuple(full)


def kernel(**inputs):
    nc = _get_nc()
    in_maps = make_in_maps(inputs)
    last_err = None
    for attempt in range(3):
        try:
            res = run_bass_kernel_spmd(nc, in_maps, core_ids=list(range(NCORES)))
            return assemble(res.results)
        except Exception as e:  # transient NRT_EXEC_UNIT_UNRECOVERABLE wedges
            last_err = e
            if attempt < 2:
                import time

                time.sleep(90)
    raise last_err

